# revision 9
# baseline (speedup 1.0000x reference)
"""MinGRU layer (LN -> gate/candidate Linear -> minGRU scan -> residual) on 8 trn2 cores.

Problem (hardcoded): x [B=4, T=4096, H=1024] fp32, weights Wg/Wc [1024,1024],
biases bg/bc [1024], LN gamma/beta [1024].

Sharding: core c = (batch b = c//2, output-half p = c%2). Every core receives
the full transposed batch row xT[b] = x[b].T (H on partitions, T on free) and
computes z/c for its 512 output channels over all T. The minGRU recurrence is
elementwise over (b, h), so with output-channel sharding each core scans its
own channels over the full sequence - no cross-core dependency, no collectives.

Per-core pipeline (all layouts [h or o on partitions, t on free]):
  1. LN is folded algebraically: gate_pre[o,t] = rstd[t]*(sum_h W'[o,h]x[h,t]
     - mu[t]*wsum[o]) + b_eff[o], with gamma/beta folded into W'/b_eff on host.
     mu/var come from ones-matmuls on the PE; the -mu*wsum term is an extra
     K=1 matmul row accumulated into the same PSUM tile.
  2. GEMMs run in float32r (full PE rate at N=512, ~1e-4 accuracy).
  3. z = sigmoid(.) on ScalarE; a = 1-z on ScalarE; b = (c_pre+bc)*z as one
     scalar_tensor_tensor on VectorE.
  4. h = tensor_tensor_scan(a, b) on VectorE, chained across 512-col chunks.
  5. out = h + xT rows (residual), DMA out; host transposes shards back.
"""

import os
import numpy as np

import concourse.bass as bass
import concourse.bacc as bacc
import concourse.tile as tile
from concourse import mybir
from concourse.bass_utils import run_bass_kernel_spmd

B, T, H = 4, 4096, 1024
EPS = 1e-5
N_CORES = 8
OH = H // 2          # output channels per core
CHUNK = 512
N_CHUNKS = T // CHUNK
KT = H // 128        # k-tiles (contraction)
OT = OH // 128       # o-tiles per core

F32 = mybir.dt.float32
F32R = mybir.dt.float32r
AF = mybir.ActivationFunctionType
OP = mybir.AluOpType

_CACHE = {}


def _build():
    nc = bacc.Bacc("TRN2", target_bir_lowering=False, debug=False)

    xT_d = nc.dram_tensor("xT", [H, T], F32R, kind="ExternalInput").ap()
    wg_d = nc.dram_tensor("wg", [H, OH], F32R, kind="ExternalInput").ap()
    wc_d = nc.dram_tensor("wc", [H, OH], F32R, kind="ExternalInput").ap()
    bg_d = nc.dram_tensor("bg", [128, OT], F32, kind="ExternalInput").ap()
    bc_d = nc.dram_tensor("bc", [128, OT], F32, kind="ExternalInput").ap()
    aug_g_d = nc.dram_tensor("aug_g", [1, OH], F32R, kind="ExternalInput").ap()
    aug_c_d = nc.dram_tensor("aug_c", [1, OH], F32R, kind="ExternalInput").ap()
    ones_d = nc.dram_tensor("ones", [128, 2], F32R, kind="ExternalInput").ap()
    onesr_d = nc.dram_tensor("onesr", [1, 128], F32R, kind="ExternalInput").ap()
    out_d = nc.dram_tensor("outT", [OH, T], F32, kind="ExternalOutput").ap()

    with tile.TileContext(nc) as tc:
        with (
            tc.tile_pool(name="const", bufs=1) as cpool,
            tc.tile_pool(name="xin", bufs=3) as xpool,
            tc.tile_pool(name="sq", bufs=2) as sqpool,
            tc.tile_pool(name="stat", bufs=2) as spool,
            tc.tile_pool(name="work", bufs=3) as wpool,
            tc.tile_pool(name="hbuf", bufs=3) as hpool,
            tc.tile_pool(name="psA", bufs=2, space="PSUM") as psA,
            tc.tile_pool(name="psB", bufs=2, space="PSUM") as psB,
            tc.tile_pool(name="psS", bufs=1, space="PSUM") as psS,
        ):
            # ---- resident constants ----
            wg_sb = cpool.tile([128, KT, OH], F32R, tag="wg")
            wc_sb = cpool.tile([128, KT, OH], F32R, tag="wc")
            nc.sync.dma_start(wg_sb[:], wg_d[:].rearrange("(k p) o -> p k o", p=128))
            nc.sync.dma_start(wc_sb[:], wc_d[:].rearrange("(k p) o -> p k o", p=128))
            bg_sb = cpool.tile([128, OT], F32, tag="bg")
            bc_sb = cpool.tile([128, OT], F32, tag="bc")
            nc.sync.dma_start(bg_sb[:], bg_d[:])
            nc.sync.dma_start(bc_sb[:], bc_d[:])
            aug_g = cpool.tile([1, OH], F32R, tag="aug_g")
            aug_c = cpool.tile([1, OH], F32R, tag="aug_c")
            nc.sync.dma_start(aug_g[:], aug_g_d[:])
            nc.sync.dma_start(aug_c[:], aug_c_d[:])
            ones_sb = cpool.tile([128, 2], F32R, tag="ones")
            nc.sync.dma_start(ones_sb[:], ones_d[:])
            onesA = ones_sb[:, 0:1]     # [128,1] lhsT for column sums
            onesR = cpool.tile([1, 128], F32R, tag="onesR")
            nc.sync.dma_start(onesR[:], onesr_d[:])

            h_prev = [None] * OT

            for i in range(N_CHUNKS):
                t0 = i * CHUNK
                # ---- load x chunk: [128, KT, CHUNK] (k-tile major on free) ----
                xc = xpool.tile([128, KT, CHUNK], F32R, tag="xc")
                nc.sync.dma_start(
                    xc[:], xT_d[:, t0 : t0 + CHUNK].rearrange("(k p) t -> p k t", p=128)
                )

                # ---- stats: col sums of x and x^2 over all H ----
                psx = psS.tile([1, CHUNK], F32, tag="psx")
                for k in range(KT):
                    nc.tensor.matmul(
                        psx[:], onesA, xc[:, k, :], start=(k == 0), stop=(k == KT - 1)
                    )
                xsq = sqpool.tile([128, KT, CHUNK], F32R, tag="xsq")
                for k in range(KT):
                    nc.scalar.activation(xsq[:, k, :], xc[:, k, :], AF.Square)
                psq = psS.tile([1, CHUNK], F32, tag="psq")
                for k in range(KT):
                    nc.tensor.matmul(
                        psq[:], onesA, xsq[:, k, :], start=(k == 0), stop=(k == KT - 1)
                    )

                mu = spool.tile([1, CHUNK], F32R, tag="mu")
                nc.scalar.mul(mu[:], psx[:], 1.0 / H)
                ex2 = spool.tile([1, CHUNK], F32, tag="ex2")
                nc.scalar.mul(ex2[:], psq[:], 1.0 / H)
                mu2 = spool.tile([1, CHUNK], F32, tag="mu2")
                nc.scalar.activation(mu2[:], mu[:], AF.Square)
                var = spool.tile([1, CHUNK], F32, tag="var")
                # var + eps = (ex2 + eps) - mu2 in one op
                nc.vector.scalar_tensor_tensor(
                    var[:], ex2[:], EPS, mu2[:], OP.add, OP.subtract
                )
                sd = spool.tile([1, CHUNK], F32, tag="sd")
                nc.scalar.activation(sd[:], var[:], AF.Sqrt)
                rstd = spool.tile([1, CHUNK], F32R, tag="rstd")
                with nc.allow_low_precision(reason="float32r for PE broadcast"):
                    nc.vector.reciprocal(rstd[:], sd[:])

                # broadcast rstd across partitions via K=1 ones matmul
                psb = psS.tile([128, CHUNK], F32, tag="psb")
                nc.tensor.matmul(psb[:], onesR[:], rstd[:], start=True, stop=True)
                rstdB = spool.tile([128, CHUNK], F32, tag="rstdB")
                nc.scalar.copy(rstdB[:], psb[:])

                # ---- per o-tile: GEMMs -> gates -> scan -> residual ----
                for o in range(OT):
                    og = o * 128
                    pg = psA.tile([128, CHUNK], F32, tag="pg")
                    for k in range(KT):
                        nc.tensor.matmul(
                            pg[:], wg_sb[:, k, og : og + 128], xc[:, k, :],
                            start=(k == 0), stop=False,
                        )
                    nc.tensor.matmul(
                        pg[:], aug_g[:, og : og + 128], mu[:], start=False, stop=True
                    )
                    pc = psB.tile([128, CHUNK], F32, tag="pc")
                    for k in range(KT):
                        nc.tensor.matmul(
                            pc[:], wc_sb[:, k, og : og + 128], xc[:, k, :],
                            start=(k == 0), stop=False,
                        )
                    nc.tensor.matmul(
                        pc[:], aug_c[:, og : og + 128], mu[:], start=False, stop=True
                    )

                    tg = wpool.tile([128, CHUNK], F32, tag="tg")
                    nc.vector.tensor_mul(tg[:], pg[:], rstdB[:])
                    z = wpool.tile([128, CHUNK], F32, tag="z")
                    nc.scalar.activation(z[:], tg[:], AF.Sigmoid, bias=bg_sb[:, o : o + 1])
                    a = wpool.tile([128, CHUNK], F32, tag="a")
                    nc.scalar.activation(a[:], z[:], AF.Copy, bias=1.0, scale=-1.0)
                    tcn = wpool.tile([128, CHUNK], F32, tag="tc")
                    nc.vector.tensor_mul(tcn[:], pc[:], rstdB[:])
                    bsc = wpool.tile([128, CHUNK], F32, tag="bsc")
                    nc.vector.scalar_tensor_tensor(
                        bsc[:], tcn[:], bc_sb[:, o : o + 1], z[:], OP.add, OP.mult
                    )

                    h = hpool.tile([128, CHUNK], F32, tag=f"h{o}")
                    init = 0.0 if i == 0 else h_prev[o][:, CHUNK - 1 : CHUNK]
                    nc.vector.tensor_tensor_scan(
                        h[:], a[:], bsc[:], init, OP.mult, OP.add
                    )
                    h_prev[o] = h

                    # xT rows are host-permuted per core so this core's
                    # residual rows always sit at k-tiles 0..OT-1
                    ot = wpool.tile([128, CHUNK], F32, tag="ot")
                    nc.vector.tensor_add(ot[:], h[:], xc[:, o, :])
                    nc.sync.dma_start(out_d[og : og + 128, t0 : t0 + CHUNK], ot[:])

    nc.compile()
    return nc


def _prep_inputs(gamma, beta, Wg, bg, Wc, bc, ohalf):
    """Host-side weight folding for one output half.

    The h-rows of the weights (and of xT, see kernel()) are rolled so this
    half's own output channels come first: the device residual then always
    reads xc k-tiles 0..OT-1 with one shared program across cores.
    """
    o0 = ohalf * OH
    perm = np.roll(np.arange(H), -o0)  # identity for half 0, swap halves for 1
    Wg_h = Wg[o0 : o0 + OH]          # [OH, H]
    Wc_h = Wc[o0 : o0 + OH]
    # lhsT layout [h, o], gamma folded into rows (h), rows permuted like xT
    wg_eff = ((Wg_h * gamma[None, :]).T)[perm].astype(np.float32)   # [H, OH]
    wc_eff = ((Wc_h * gamma[None, :]).T)[perm].astype(np.float32)
    bg_eff = (bg[o0 : o0 + OH] + Wg_h @ beta).astype(np.float32)
    bc_eff = (bc[o0 : o0 + OH] + Wc_h @ beta).astype(np.float32)
    wsum_g = wg_eff.sum(axis=0).astype(np.float32)          # [OH]
    wsum_c = wc_eff.sum(axis=0).astype(np.float32)
    return {
        "wg": np.ascontiguousarray(wg_eff),
        "wc": np.ascontiguousarray(wc_eff),
        "bg": np.ascontiguousarray(bg_eff.reshape(OT, 128).T),
        "bc": np.ascontiguousarray(bc_eff.reshape(OT, 128).T),
        "aug_g": np.ascontiguousarray(-wsum_g[None, :]),
        "aug_c": np.ascontiguousarray(-wsum_c[None, :]),
        "ones": np.ones((128, 2), dtype=np.float32),
        "onesr": np.ones((1, 128), dtype=np.float32),
    }


def kernel(x, gamma, beta, Wg, bg, Wc, bc):
    x = np.asarray(x, dtype=np.float32)
    gamma = np.asarray(gamma, dtype=np.float32)
    beta = np.asarray(beta, dtype=np.float32)
    Wg = np.asarray(Wg, dtype=np.float32)
    bg = np.asarray(bg, dtype=np.float32)
    Wc = np.asarray(Wc, dtype=np.float32)
    bc = np.asarray(bc, dtype=np.float32)

    if "nc" not in _CACHE:
        _CACHE["nc"] = _build()
    nc = _CACHE["nc"]

    xT = [np.ascontiguousarray(x[b].T) for b in range(B)]  # [H, T] each
    halves = [_prep_inputs(gamma, beta, Wg, bg, Wc, bc, p) for p in range(2)]

    in_maps = []
    for c in range(N_CORES):
        b, p = divmod(c, 2)
        m = dict(halves[p])
        # roll h-rows to match the weight-row permutation for this half
        m["xT"] = xT[b] if p == 0 else np.ascontiguousarray(np.roll(xT[b], -OH, axis=0))
        in_maps.append(m)

    trace = bool(int(os.environ.get("MINGRU_TRACE", "0")))
    kwargs = {}
    if trace:
        tmpdir = os.environ.get("MINGRU_TRACE_DIR") or None
        kwargs = dict(trace=True, tmpdir=tmpdir)
    res = run_bass_kernel_spmd(nc, in_maps, core_ids=list(range(N_CORES)), **kwargs)
    if trace:
        _CACHE["last_results"] = res

    out = np.empty((B, T, H), dtype=np.float32)
    for c in range(N_CORES):
        b, p = divmod(c, 2)
        out[b, :, p * OH : (p + 1) * OH] = res.results[c]["outT"].T
    return out


# revision 10
# speedup vs baseline: 1.0540x; 1.0540x over previous
"""MinGRU layer (LN -> gate/candidate Linear -> minGRU scan -> residual) on 8 trn2 cores.

Problem (hardcoded): x [B=4, T=4096, H=1024] fp32, weights Wg/Wc [1024,1024],
biases bg/bc [1024], LN gamma/beta [1024].

Sharding: core c = (batch b = c//2, output-half p = c%2). Every core receives
the full transposed batch row xT[b] = x[b].T (H on partitions, T on free) and
computes z/c for its 512 output channels over all T. The minGRU recurrence is
elementwise over (b, h), so with output-channel sharding each core scans its
own channels over the full sequence - no cross-core dependency, no collectives.

Per-core pipeline (all layouts [h or o on partitions, t on free]):
  1. LN folded algebraically: gate_pre[o,t] = rstd[t]*(sum_h W'[o,h]x[h,t]
     - mu[t]*wsum[o]) + b_eff[o], with gamma/beta folded into W'/b_eff on host.
     mu/var come from ones-matmuls on the PE; the -mu*wsum term is an extra
     K=1 matmul row accumulated into the same PSUM tile.
  2. GEMMs run in bf16 (fp32 PSUM accumulate). fp32/fp32r would force a
     non-overlapped 187ns LDWEIGHTS per matmul (no FWL, no background load);
     bf16 hides the weight load and streams 512 cols at 2.4GHz.
  3. rstd = exp(-0.5*ln(var+eps)) on ScalarE (vector.reciprocal is an 8x
     iterative divide; Rsqrt activation is banned for accuracy).
  4. z = sigmoid(pre); a = 1-z computed as sigmoid(-pre) (second ACT pass,
     no dependency on z). b = (c_pre+bc)*z as one scalar_tensor_tensor.
  5. h = tensor_tensor_scan(a, b) on VectorE, chained across 512-col chunks.
  6. out = h + x rows (fp32 residual input, separate from the bf16 GEMM x),
     on GpSimd to offload VectorE; DMA out; host transposes shards back.
"""

import os
import numpy as np
import ml_dtypes

import concourse.bass as bass
import concourse.bacc as bacc
import concourse.tile as tile
from concourse import mybir
from concourse.bass_utils import run_bass_kernel_spmd

B, T, H = 4, 4096, 1024
EPS = 1e-5
N_CORES = 8
OH = H // 2          # output channels per core
CHUNK = 512
N_CHUNKS = T // CHUNK
KT = H // 128        # k-tiles (contraction)
OT = OH // 128       # o-tiles per core

F32 = mybir.dt.float32
BF16 = mybir.dt.bfloat16
AF = mybir.ActivationFunctionType
OP = mybir.AluOpType
BF = ml_dtypes.bfloat16

_CACHE = {}


def _build():
    nc = bacc.Bacc("TRN2", target_bir_lowering=False, debug=False)

    xT_d = nc.dram_tensor("xT", [H, T], BF16, kind="ExternalInput").ap()
    xr_d = nc.dram_tensor("xr", [OH, T], F32, kind="ExternalInput").ap()
    wg_d = nc.dram_tensor("wg", [H, OH], BF16, kind="ExternalInput").ap()
    wc_d = nc.dram_tensor("wc", [H, OH], BF16, kind="ExternalInput").ap()
    bg_d = nc.dram_tensor("bg", [128, OT], F32, kind="ExternalInput").ap()
    bgn_d = nc.dram_tensor("bgn", [128, OT], F32, kind="ExternalInput").ap()
    bc_d = nc.dram_tensor("bc", [128, OT], F32, kind="ExternalInput").ap()
    aug_g_d = nc.dram_tensor("aug_g", [1, OH], BF16, kind="ExternalInput").ap()
    aug_c_d = nc.dram_tensor("aug_c", [1, OH], BF16, kind="ExternalInput").ap()
    ones_d = nc.dram_tensor("ones", [128, 2], BF16, kind="ExternalInput").ap()
    onesr_d = nc.dram_tensor("onesr", [1, 128], BF16, kind="ExternalInput").ap()
    out_d = nc.dram_tensor("outT", [OH, T], F32, kind="ExternalOutput").ap()

    with tile.TileContext(nc) as tc:
        with (
            tc.tile_pool(name="const", bufs=1) as cpool,
            tc.tile_pool(name="xin", bufs=3) as xpool,
            tc.tile_pool(name="sq", bufs=2) as sqpool,
            tc.tile_pool(name="stat", bufs=2) as spool,
            tc.tile_pool(name="work", bufs=3) as wpool,
            tc.tile_pool(name="hbuf", bufs=3) as hpool,
            tc.tile_pool(name="psA", bufs=2, space="PSUM") as psA,
            tc.tile_pool(name="psB", bufs=2, space="PSUM") as psB,
            tc.tile_pool(name="psS", bufs=1, space="PSUM") as psS,
        ):
            # ---- resident constants ----
            wg_sb = cpool.tile([128, KT, OH], BF16, tag="wg")
            wc_sb = cpool.tile([128, KT, OH], BF16, tag="wc")
            nc.sync.dma_start(wg_sb[:], wg_d[:].rearrange("(k p) o -> p k o", p=128))
            nc.sync.dma_start(wc_sb[:], wc_d[:].rearrange("(k p) o -> p k o", p=128))
            bg_sb = cpool.tile([128, OT], F32, tag="bg")
            bgn_sb = cpool.tile([128, OT], F32, tag="bgn")
            bc_sb = cpool.tile([128, OT], F32, tag="bc")
            nc.sync.dma_start(bg_sb[:], bg_d[:])
            nc.sync.dma_start(bgn_sb[:], bgn_d[:])
            nc.sync.dma_start(bc_sb[:], bc_d[:])
            aug_g = cpool.tile([1, OH], BF16, tag="aug_g")
            aug_c = cpool.tile([1, OH], BF16, tag="aug_c")
            nc.sync.dma_start(aug_g[:], aug_g_d[:])
            nc.sync.dma_start(aug_c[:], aug_c_d[:])
            ones_sb = cpool.tile([128, 2], BF16, tag="ones")
            nc.sync.dma_start(ones_sb[:], ones_d[:])
            onesA = ones_sb[:, 0:1]     # [128,1] lhsT for column sums
            onesR = cpool.tile([1, 128], BF16, tag="onesR")
            nc.sync.dma_start(onesR[:], onesr_d[:])

            h_prev = [None] * OT

            for i in range(N_CHUNKS):
                t0 = i * CHUNK
                # ---- load x chunk (bf16 GEMM copy + fp32 residual rows) ----
                xc = xpool.tile([128, KT, CHUNK], BF16, tag="xc")
                nc.sync.dma_start(
                    xc[:], xT_d[:, t0 : t0 + CHUNK].rearrange("(k p) t -> p k t", p=128)
                )
                xrc = xpool.tile([128, OT, CHUNK], F32, tag="xrc")
                nc.sync.dma_start(
                    xrc[:], xr_d[:, t0 : t0 + CHUNK].rearrange("(k p) t -> p k t", p=128)
                )

                # ---- stats: col sums of x and x^2 over all H (PE) ----
                psx = psS.tile([1, CHUNK], F32, tag="psx")
                for k in range(KT):
                    nc.tensor.matmul(
                        psx[:], onesA, xc[:, k, :], start=(k == 0), stop=(k == KT - 1)
                    )
                xsq = sqpool.tile([128, KT, CHUNK], BF16, tag="xsq")
                for k in range(KT):
                    nc.scalar.activation(xsq[:, k, :], xc[:, k, :], AF.Square)
                psq = psS.tile([1, CHUNK], F32, tag="psq")
                for k in range(KT):
                    nc.tensor.matmul(
                        psq[:], onesA, xsq[:, k, :], start=(k == 0), stop=(k == KT - 1)
                    )

                # ---- mu, var, rstd = exp(-0.5 ln(var+eps)) ----
                mu = spool.tile([1, CHUNK], BF16, tag="mu")
                nc.vector.tensor_scalar_mul(mu[:], psx[:], 1.0 / H)
                ex2 = spool.tile([1, CHUNK], F32, tag="ex2")
                nc.vector.tensor_scalar_mul(ex2[:], psq[:], 1.0 / H)
                mu2 = spool.tile([1, CHUNK], F32, tag="mu2")
                nc.vector.tensor_mul(mu2[:], mu[:], mu[:])
                var = spool.tile([1, CHUNK], F32, tag="var")
                nc.vector.scalar_tensor_tensor(
                    var[:], ex2[:], EPS, mu2[:], OP.add, OP.subtract
                )
                lnv = spool.tile([1, CHUNK], F32, tag="lnv")
                nc.scalar.activation(lnv[:], var[:], AF.Ln)
                rstd = spool.tile([1, CHUNK], BF16, tag="rstd")
                with nc.allow_low_precision(reason="bf16 rstd for PE broadcast"):
                    nc.scalar.activation(rstd[:], lnv[:], AF.Exp, scale=-0.5)

                # broadcast rstd across partitions via K=1 ones matmul
                psb = psS.tile([128, CHUNK], F32, tag="psb")
                nc.tensor.matmul(psb[:], onesR[:], rstd[:], start=True, stop=True)
                rstdB = spool.tile([128, CHUNK], F32, tag="rstdB")
                nc.scalar.copy(rstdB[:], psb[:])

                # ---- per o-tile: GEMMs -> gates -> scan -> residual ----
                for o in range(OT):
                    og = o * 128
                    pg = psA.tile([128, CHUNK], F32, tag="pg")
                    for k in range(KT):
                        nc.tensor.matmul(
                            pg[:], wg_sb[:, k, og : og + 128], xc[:, k, :],
                            start=(k == 0), stop=False,
                        )
                    nc.tensor.matmul(
                        pg[:], aug_g[:, og : og + 128], mu[:], start=False, stop=True
                    )
                    pc = psB.tile([128, CHUNK], F32, tag="pc")
                    for k in range(KT):
                        nc.tensor.matmul(
                            pc[:], wc_sb[:, k, og : og + 128], xc[:, k, :],
                            start=(k == 0), stop=False,
                        )
                    nc.tensor.matmul(
                        pc[:], aug_c[:, og : og + 128], mu[:], start=False, stop=True
                    )

                    tg = wpool.tile([128, CHUNK], F32, tag="tg")
                    nc.vector.tensor_mul(tg[:], pg[:], rstdB[:])
                    z = wpool.tile([128, CHUNK], F32, tag="z")
                    nc.scalar.activation(z[:], tg[:], AF.Sigmoid, bias=bg_sb[:, o : o + 1])
                    # a = 1 - z = sigmoid(-(pre + bg)) -- independent of z
                    a = wpool.tile([128, CHUNK], F32, tag="a")
                    nc.scalar.activation(
                        a[:], tg[:], AF.Sigmoid, bias=bgn_sb[:, o : o + 1], scale=-1.0
                    )
                    tcn = wpool.tile([128, CHUNK], F32, tag="tc")
                    nc.vector.tensor_mul(tcn[:], pc[:], rstdB[:])
                    bsc = wpool.tile([128, CHUNK], F32, tag="bsc")
                    nc.vector.scalar_tensor_tensor(
                        bsc[:], tcn[:], bc_sb[:, o : o + 1], z[:], OP.add, OP.mult
                    )

                    h = hpool.tile([128, CHUNK], F32, tag=f"h{o}")
                    init = 0.0 if i == 0 else h_prev[o][:, CHUNK - 1 : CHUNK]
                    nc.vector.tensor_tensor_scan(
                        h[:], a[:], bsc[:], init, OP.mult, OP.add
                    )
                    h_prev[o] = h

                    ot = wpool.tile([128, CHUNK], F32, tag="ot")
                    nc.gpsimd.tensor_add(ot[:], h[:], xrc[:, o, :])
                    nc.sync.dma_start(out_d[og : og + 128, t0 : t0 + CHUNK], ot[:])

    nc.compile()
    return nc


def _prep_inputs(gamma, beta, Wg, bg, Wc, bc, ohalf):
    """Host-side weight folding for one output half.

    The h-rows of the weights (and of xT, see kernel()) are rolled so this
    half's own output channels come first: the device residual then always
    reads x rows at k-tiles 0..OT-1 with one shared program across cores.
    """
    o0 = ohalf * OH
    perm = np.roll(np.arange(H), -o0)  # identity for half 0, swap halves for 1
    Wg_h = Wg[o0 : o0 + OH]          # [OH, H]
    Wc_h = Wc[o0 : o0 + OH]
    # lhsT layout [h, o], gamma folded into rows (h), rows permuted like xT
    wg_eff = ((Wg_h * gamma[None, :]).T)[perm].astype(np.float32)   # [H, OH]
    wc_eff = ((Wc_h * gamma[None, :]).T)[perm].astype(np.float32)
    bg_eff = (bg[o0 : o0 + OH] + Wg_h @ beta).astype(np.float32)
    bc_eff = (bc[o0 : o0 + OH] + Wc_h @ beta).astype(np.float32)
    # wsum must match what the device GEMM actually sums: bf16 weights
    wg_bf = wg_eff.astype(BF)
    wc_bf = wc_eff.astype(BF)
    wsum_g = wg_bf.astype(np.float32).sum(axis=0)
    wsum_c = wc_bf.astype(np.float32).sum(axis=0)
    return {
        "wg": np.ascontiguousarray(wg_bf),
        "wc": np.ascontiguousarray(wc_bf),
        "bg": np.ascontiguousarray(bg_eff.reshape(OT, 128).T),
        "bgn": np.ascontiguousarray(-bg_eff.reshape(OT, 128).T),
        "bc": np.ascontiguousarray(bc_eff.reshape(OT, 128).T),
        "aug_g": np.ascontiguousarray(-wsum_g[None, :].astype(BF)),
        "aug_c": np.ascontiguousarray(-wsum_c[None, :].astype(BF)),
        "ones": np.ones((128, 2), dtype=BF),
        "onesr": np.ones((1, 128), dtype=BF),
    }


def kernel(x, gamma, beta, Wg, bg, Wc, bc):
    x = np.asarray(x, dtype=np.float32)
    gamma = np.asarray(gamma, dtype=np.float32)
    beta = np.asarray(beta, dtype=np.float32)
    Wg = np.asarray(Wg, dtype=np.float32)
    bg = np.asarray(bg, dtype=np.float32)
    Wc = np.asarray(Wc, dtype=np.float32)
    bc = np.asarray(bc, dtype=np.float32)

    if "nc" not in _CACHE:
        _CACHE["nc"] = _build()
    nc = _CACHE["nc"]

    xT = [np.ascontiguousarray(x[b].T) for b in range(B)]  # [H, T] each
    halves = [_prep_inputs(gamma, beta, Wg, bg, Wc, bc, p) for p in range(2)]

    in_maps = []
    for c in range(N_CORES):
        b, p = divmod(c, 2)
        m = dict(halves[p])
        # roll h-rows to match the weight-row permutation for this half
        xr = xT[b] if p == 0 else np.roll(xT[b], -OH, axis=0)
        m["xT"] = np.ascontiguousarray(xr.astype(BF))
        m["xr"] = np.ascontiguousarray(xr[:OH])
        in_maps.append(m)

    trace = bool(int(os.environ.get("MINGRU_TRACE", "0")))
    kwargs = {}
    if trace:
        tmpdir = os.environ.get("MINGRU_TRACE_DIR") or None
        kwargs = dict(trace=True, tmpdir=tmpdir)
    res = run_bass_kernel_spmd(nc, in_maps, core_ids=list(range(N_CORES)), **kwargs)
    if trace:
        _CACHE["last_results"] = res

    out = np.empty((B, T, H), dtype=np.float32)
    for c in range(N_CORES):
        b, p = divmod(c, 2)
        out[b, :, p * OH : (p + 1) * OH] = res.results[c]["outT"].T
    return out


# revision 15
# speedup vs baseline: 1.1126x; 1.0556x over previous
"""MinGRU layer (LN -> gate/candidate Linear -> minGRU scan -> residual) on 8 trn2 cores.

Problem (hardcoded): x [B=4, T=4096, H=1024] fp32, weights Wg/Wc [1024,1024],
biases bg/bc [1024], LN gamma/beta [1024].

Sharding: core c = (batch b = c//2, output-half p = c%2). Every core receives
the full transposed batch row xT[b] = x[b].T (H on partitions, T on free) and
computes z/c for its 512 output channels over all T. The minGRU recurrence is
elementwise over (b, h), so with output-channel sharding each core scans its
own channels over the full sequence - no cross-core dependency, no collectives.

Per-core pipeline (layouts [h or o on partitions, t on free], 512-col chunks,
stats for chunk i+1 software-pipelined under the GEMMs of chunk i):
  1. LN folded algebraically: gate_pre[o,t] = sum_h W'[o,h]*(x[h,t]*rstd[t])
     - (mu*rstd)[t]*wsum[o] + b_eff[o], gamma/beta folded into W'/b_eff on
     host. mu/var from ones-matmuls on PE; x*rstd pre-scaled on VectorE in
     bf16 2x mode; the -mu*rstd*wsum term is a K=1 matmul row into the same
     PSUM tile; sigmoids read PSUM directly with per-partition bias.
  2. GEMMs in bf16 (fp32 PSUM). fp32/fp32r would force a non-overlapped
     ~187ns LDWEIGHTS per matmul; bf16 hides the weight load.
  3. rstd = exp(-0.5*ln(var+eps)) on ScalarE (vector.reciprocal is an 8x
     iterative divide; Rsqrt activation is banned for accuracy). Square/Copy/
     Sigmoid share one ACT table set; only Ln/Exp force 2 set switches/chunk.
  4. z = sigmoid(pre+bg); a = 1-z as sigmoid(-pre-bg) (independent of z);
     b = (c_pre+bc)*z as one scalar_tensor_tensor.
  5. h = tensor_tensor_scan(a, b) on VectorE, chained across chunks.
  6. out = h + x rows (fp32 residual input, separate from the bf16 GEMM x),
     on GpSimd; DMA out; host transposes shards back.
"""

import os
import numpy as np
import ml_dtypes

import concourse.bass as bass
import concourse.bacc as bacc
import concourse.tile as tile
from concourse import mybir
from concourse.bass_utils import run_bass_kernel_spmd

B, T, H = 4, 4096, 1024
EPS = 1e-5
N_CORES = 8
OH = H // 2          # output channels per core
CHUNK = 512
N_CHUNKS = T // CHUNK
KT = H // 128        # k-tiles (contraction)
OT = OH // 128       # o-tiles per core

F32 = mybir.dt.float32
BF16 = mybir.dt.bfloat16
AF = mybir.ActivationFunctionType
OP = mybir.AluOpType
BF = ml_dtypes.bfloat16

_CACHE = {}


def _build():
    nc = bacc.Bacc("TRN2", target_bir_lowering=False, debug=False)

    xT_d = nc.dram_tensor("xT", [H, T], BF16, kind="ExternalInput").ap()
    xr_d = nc.dram_tensor("xr", [OH, T], F32, kind="ExternalInput").ap()
    wg_d = nc.dram_tensor("wg", [H, OH], BF16, kind="ExternalInput").ap()
    wc_d = nc.dram_tensor("wc", [H, OH], BF16, kind="ExternalInput").ap()
    bg_d = nc.dram_tensor("bg", [128, OT], F32, kind="ExternalInput").ap()
    bgn_d = nc.dram_tensor("bgn", [128, OT], F32, kind="ExternalInput").ap()
    bc_d = nc.dram_tensor("bc", [128, OT], F32, kind="ExternalInput").ap()
    aug_g_d = nc.dram_tensor("aug_g", [1, OH], BF16, kind="ExternalInput").ap()
    aug_c_d = nc.dram_tensor("aug_c", [1, OH], BF16, kind="ExternalInput").ap()
    ones_d = nc.dram_tensor("ones", [128, 2], BF16, kind="ExternalInput").ap()
    onesr_d = nc.dram_tensor("onesr", [1, 128], BF16, kind="ExternalInput").ap()
    out_d = nc.dram_tensor("outT", [OH, T], F32, kind="ExternalOutput").ap()

    with tile.TileContext(nc) as tc:
        with (
            tc.tile_pool(name="const", bufs=1) as cpool,
            tc.tile_pool(name="xin", bufs=3) as xpool,
            tc.tile_pool(name="sq", bufs=2) as sqpool,
            tc.tile_pool(name="xnp", bufs=2) as xnpool,
            tc.tile_pool(name="stat", bufs=2) as spool,
            tc.tile_pool(name="work", bufs=3) as wpool,
            tc.tile_pool(name="hbuf", bufs=3) as hpool,
            tc.tile_pool(name="psA", bufs=2, space="PSUM") as psA,
            tc.tile_pool(name="psB", bufs=2, space="PSUM") as psB,
            tc.tile_pool(name="psS", bufs=2, space="PSUM") as psS,
            tc.tile_pool(name="psb", bufs=1, space="PSUM") as psbp,
        ):
            # ---- resident constants ----
            wg_sb = cpool.tile([128, KT, OH], BF16, tag="wg")
            wc_sb = cpool.tile([128, KT, OH], BF16, tag="wc")
            nc.sync.dma_start(wg_sb[:], wg_d[:].rearrange("(k p) o -> p k o", p=128))
            nc.sync.dma_start(wc_sb[:], wc_d[:].rearrange("(k p) o -> p k o", p=128))
            bg_sb = cpool.tile([128, OT], F32, tag="bg")
            bgn_sb = cpool.tile([128, OT], F32, tag="bgn")
            bc_sb = cpool.tile([128, OT], F32, tag="bc")
            nc.sync.dma_start(bg_sb[:], bg_d[:])
            nc.sync.dma_start(bgn_sb[:], bgn_d[:])
            nc.sync.dma_start(bc_sb[:], bc_d[:])
            aug_g = cpool.tile([1, OH], BF16, tag="aug_g")
            aug_c = cpool.tile([1, OH], BF16, tag="aug_c")
            nc.sync.dma_start(aug_g[:], aug_g_d[:])
            nc.sync.dma_start(aug_c[:], aug_c_d[:])
            ones_sb = cpool.tile([128, 2], BF16, tag="ones")
            nc.sync.dma_start(ones_sb[:], ones_d[:])
            onesA = ones_sb[:, 0:1]     # [128,1] lhsT for column sums
            onesR = cpool.tile([1, 128], BF16, tag="onesR")
            nc.sync.dma_start(onesR[:], onesr_d[:])

            h_prev = [None] * OT
            xc_t = [None] * N_CHUNKS
            stat_t = [None] * N_CHUNKS   # (muex, rstd, mr)

            def load_x(i):
                t0 = i * CHUNK
                xc = xpool.tile([128, KT, CHUNK], BF16, tag="xc")
                nc.sync.dma_start(
                    xc[:], xT_d[:, t0 : t0 + CHUNK].rearrange("(k p) t -> p k t", p=128)
                )
                xc_t[i] = xc

            def emit_stats(i):
                """Column sums + mu/var/rstd chain for chunk i."""
                xc = xc_t[i]
                st = psS.tile([33, CHUNK], F32, tag="st")
                for k in range(KT):
                    nc.tensor.matmul(
                        st[0:1, :], onesA, xc[:, k, :],
                        start=(k == 0), stop=(k == KT - 1),
                    )
                xsq = sqpool.tile([128, KT, CHUNK], BF16, tag="xsq")
                for k in range(KT):
                    if k < 4:
                        nc.scalar.activation(xsq[:, k, :], xc[:, k, :], AF.Square)
                    else:
                        nc.gpsimd.tensor_mul(xsq[:, k, :], xc[:, k, :], xc[:, k, :])
                for k in range(KT):
                    nc.tensor.matmul(
                        st[32:33, :], onesA, xsq[:, k, :],
                        start=(k == 0), stop=(k == KT - 1),
                    )

                mu = spool.tile([1, CHUNK], F32, tag="mu")
                nc.scalar.mul(mu[:], st[0:1, :], 1.0 / H)
                ex2 = spool.tile([1, CHUNK], F32, tag="ex2")
                nc.scalar.mul(ex2[:], st[32:33, :], 1.0 / H)
                mu2 = spool.tile([1, CHUNK], F32, tag="mu2")
                nc.scalar.activation(mu2[:], mu[:], AF.Square)
                var = spool.tile([1, CHUNK], F32, tag="var")
                nc.vector.scalar_tensor_tensor(
                    var[:], ex2[:], EPS, mu2[:], OP.add, OP.subtract
                )
                lnv = spool.tile([1, CHUNK], F32, tag="lnv")
                nc.scalar.activation(lnv[:], var[:], AF.Ln)
                rstd = spool.tile([1, CHUNK], BF16, tag="rstd")
                with nc.allow_low_precision(reason="bf16 rstd for bf16 GEMM prescale"):
                    nc.scalar.activation(rstd[:], lnv[:], AF.Exp, scale=-0.5)
                mr = spool.tile([1, CHUNK], BF16, tag="mr")
                nc.vector.tensor_mul(mr[:], mu[:], rstd[:])
                stat_t[i] = (rstd, mr)

            def emit_chunk(i):
                """Broadcast rstd, pre-scale x, GEMMs, gates, scan, residual."""
                t0 = i * CHUNK
                xc = xc_t[i]
                rstd, mr = stat_t[i]

                xrc = xpool.tile([128, OT, CHUNK], F32, tag="xrc")
                nc.sync.dma_start(
                    xrc[:], xr_d[:, t0 : t0 + CHUNK].rearrange("(k p) t -> p k t", p=128)
                )

                psb = psbp.tile([128, CHUNK], F32, tag="psb")
                nc.tensor.matmul(psb[:], onesR[:], rstd[:], start=True, stop=True)
                rstdB = spool.tile([128, CHUNK], BF16, tag="rstdB")
                with nc.allow_low_precision(reason="bf16 rstd broadcast"):
                    nc.vector.tensor_scalar_mul(rstdB[:], psb[:], 1.0)

                xn = xnpool.tile([128, KT, CHUNK], BF16, tag="xn")
                for k in range(KT):
                    nc.vector.tensor_mul(xn[:, k, :], xc[:, k, :], rstdB[:])

                for o in range(OT):
                    og = o * 128
                    pg = psA.tile([128, CHUNK], F32, tag="pg")
                    for k in range(KT):
                        nc.tensor.matmul(
                            pg[:], wg_sb[:, k, og : og + 128], xn[:, k, :],
                            start=(k == 0), stop=False,
                        )
                    nc.tensor.matmul(
                        pg[:], aug_g[:, og : og + 128], mr[:], start=False, stop=True
                    )
                    pc = psB.tile([128, CHUNK], F32, tag="pc")
                    for k in range(KT):
                        nc.tensor.matmul(
                            pc[:], wc_sb[:, k, og : og + 128], xn[:, k, :],
                            start=(k == 0), stop=False,
                        )
                    nc.tensor.matmul(
                        pc[:], aug_c[:, og : og + 128], mr[:], start=False, stop=True
                    )

                    z = wpool.tile([128, CHUNK], F32, tag="z")
                    nc.scalar.activation(z[:], pg[:], AF.Sigmoid, bias=bg_sb[:, o : o + 1])
                    # a = 1 - z = sigmoid(-(pre + bg)) -- independent of z
                    a = wpool.tile([128, CHUNK], F32, tag="a")
                    nc.scalar.activation(
                        a[:], pg[:], AF.Sigmoid, bias=bgn_sb[:, o : o + 1], scale=-1.0
                    )
                    bsc = wpool.tile([128, CHUNK], F32, tag="bsc")
                    nc.vector.scalar_tensor_tensor(
                        bsc[:], pc[:], bc_sb[:, o : o + 1], z[:], OP.add, OP.mult
                    )

                    h = hpool.tile([128, CHUNK], F32, tag=f"h{o}")
                    init = 0.0 if i == 0 else h_prev[o][:, CHUNK - 1 : CHUNK]
                    nc.vector.tensor_tensor_scan(
                        h[:], a[:], bsc[:], init, OP.mult, OP.add
                    )
                    h_prev[o] = h

                    ot = wpool.tile([128, CHUNK], F32, tag="ot")
                    nc.gpsimd.tensor_add(ot[:], h[:], xrc[:, o, :])
                    nc.sync.dma_start(out_d[og : og + 128, t0 : t0 + CHUNK], ot[:])

            # ---- software pipeline: stats for i+1 run under the GEMMs of i ----
            load_x(0)
            emit_stats(0)
            for i in range(N_CHUNKS):
                if i + 1 < N_CHUNKS:
                    load_x(i + 1)
                    emit_stats(i + 1)
                emit_chunk(i)

    nc.compile()
    return nc


def _prep_inputs(gamma, beta, Wg, bg, Wc, bc, ohalf):
    """Host-side weight folding for one output half.

    The h-rows of the weights (and of xT, see kernel()) are rolled so this
    half's own output channels come first: the device residual then always
    reads x rows at k-tiles 0..OT-1 with one shared program across cores.
    """
    o0 = ohalf * OH
    perm = np.roll(np.arange(H), -o0)  # identity for half 0, swap halves for 1
    Wg_h = Wg[o0 : o0 + OH]          # [OH, H]
    Wc_h = Wc[o0 : o0 + OH]
    # lhsT layout [h, o], gamma folded into rows (h), rows permuted like xT
    wg_eff = ((Wg_h * gamma[None, :]).T)[perm].astype(np.float32)   # [H, OH]
    wc_eff = ((Wc_h * gamma[None, :]).T)[perm].astype(np.float32)
    bg_eff = (bg[o0 : o0 + OH] + Wg_h @ beta).astype(np.float32)
    bc_eff = (bc[o0 : o0 + OH] + Wc_h @ beta).astype(np.float32)
    # wsum must match what the device GEMM actually sums: bf16 weights
    wg_bf = wg_eff.astype(BF)
    wc_bf = wc_eff.astype(BF)
    wsum_g = wg_bf.astype(np.float32).sum(axis=0)
    wsum_c = wc_bf.astype(np.float32).sum(axis=0)
    return {
        "wg": np.ascontiguousarray(wg_bf),
        "wc": np.ascontiguousarray(wc_bf),
        "bg": np.ascontiguousarray(bg_eff.reshape(OT, 128).T),
        "bgn": np.ascontiguousarray(-bg_eff.reshape(OT, 128).T),
        "bc": np.ascontiguousarray(bc_eff.reshape(OT, 128).T),
        "aug_g": np.ascontiguousarray(-wsum_g[None, :].astype(BF)),
        "aug_c": np.ascontiguousarray(-wsum_c[None, :].astype(BF)),
        "ones": np.ones((128, 2), dtype=BF),
        "onesr": np.ones((1, 128), dtype=BF),
    }


def kernel(x, gamma, beta, Wg, bg, Wc, bc):
    x = np.asarray(x, dtype=np.float32)
    gamma = np.asarray(gamma, dtype=np.float32)
    beta = np.asarray(beta, dtype=np.float32)
    Wg = np.asarray(Wg, dtype=np.float32)
    bg = np.asarray(bg, dtype=np.float32)
    Wc = np.asarray(Wc, dtype=np.float32)
    bc = np.asarray(bc, dtype=np.float32)

    if "nc" not in _CACHE:
        _CACHE["nc"] = _build()
    nc = _CACHE["nc"]

    xT = [np.ascontiguousarray(x[b].T) for b in range(B)]  # [H, T] each
    halves = [_prep_inputs(gamma, beta, Wg, bg, Wc, bc, p) for p in range(2)]

    in_maps = []
    for c in range(N_CORES):
        b, p = divmod(c, 2)
        m = dict(halves[p])
        # roll h-rows to match the weight-row permutation for this half
        xr = xT[b] if p == 0 else np.roll(xT[b], -OH, axis=0)
        m["xT"] = np.ascontiguousarray(xr.astype(BF))
        m["xr"] = np.ascontiguousarray(xr[:OH])
        in_maps.append(m)

    trace = bool(int(os.environ.get("MINGRU_TRACE", "0")))
    kwargs = {}
    if trace:
        tmpdir = os.environ.get("MINGRU_TRACE_DIR") or None
        kwargs = dict(trace=True, tmpdir=tmpdir)
    res = run_bass_kernel_spmd(nc, in_maps, core_ids=list(range(N_CORES)), **kwargs)
    if trace:
        _CACHE["last_results"] = res

    out = np.empty((B, T, H), dtype=np.float32)
    for c in range(N_CORES):
        b, p = divmod(c, 2)
        out[b, :, p * OH : (p + 1) * OH] = res.results[c]["outT"].T
    return out
